# revision 17
# baseline (speedup 1.0000x reference)
"""Trainium2 Bass kernel for CustomPositionsPiecewiseConv2d.

Math: for knots [-1,-.5,0,.5,1] and x in [0,1] the active coefficients are
c2 = relu(1-2v), c4 = relu(2v-1), c3 = 1-c2-c4.  Folding c3 away and then
eliminating c2 via the identity c2 - c4 = 1 - 2v gives a TWO-plane GEMM:

    out = sum_ck  g(v) * A'[c,k,o]  +  v * B[c,k,o]  + bias_eff[o]
    g(v) = relu(v - 0.5)            (= c4/2)
    A'   = 2*(W2 + W4 - 2*W3)
    B    = -2*(W2 - W3)
    bias_eff = bias + sum_ck W2

The v-plane is just (padded, bf16-cast) x — no compute — and g needs one
activation op.  Both planes are 0 on the zero-padding border.

Tap packing: Y holds [v, g, v-shift-up-1row, g-shift-up-1row] across 128
partitions, so one K=128 matmul covers taps (0,kw) and (1,kw) at once.
Taps (2,0)/(2,1) run as two concurrent K=64 row-tiled matmuls (array rows
0-63 / 64-127); tap (2,2) is a lone K=64 matmul.  5 matmul slots per
output tile instead of 9 (theoretical floor 4.5).

Everything is bf16 on the PE (err ~3e-3 << the 2e-2 gate); PSUM stays f32.
x is cast to bf16 on the host; output DMAs back as bf16 and is cast to f32
on the host (halves HBM traffic both ways).

Sharding: data-parallel over batch, 2 images per core on 8 cores.
"""

import numpy as np

B, C, H, W = 16, 32, 64, 64
O, P, KH, KW = 128, 5, 3, 3
NCORES = 8
IPC = B // NCORES            # images per core
HP, WP = H + 2, W + 2        # padded image (pad=1)
RT = 8                       # output rows per L-tile
NT = H // RT                 # L-tiles per image
K2 = KH * KW
ATOL = 1e-5
RTOL = 1e-5
ROWTILE = False


# ---------------------------------------------------------------- host math


def _isclose_np(a, b):
    return np.abs(a - b) <= np.float32(ATOL) + np.float32(RTOL) * np.abs(b)


def _reference_np(x, weights, bias, positions):
    """Direct numpy port of the reference (fallback path)."""
    EPS = 1e-6
    Bn, Cn, Hn, Wn = x.shape
    On, _, Pn, KHn, KWn = weights.shape
    xp = np.pad(x, ((0, 0), (0, 0), (1, 1), (1, 1)))
    cols = [
        xp[:, :, i : i + Hn, j : j + Wn] for i in range(KHn) for j in range(KWn)
    ]
    pat = np.stack(cols, axis=2)
    v = pat.reshape(Bn, Cn, KHn * KWn, Hn * Wn).astype(np.float32)

    left, right = positions[:-1], positions[1:]
    denom = right - left
    denom = np.where(denom == 0, np.float32(EPS), denom)
    varc = (1.0 / denom).astype(np.float32)
    const = (-left * varc).astype(np.float32)

    m_first = _isclose_np(v, positions[0])
    m_last = _isclose_np(v, positions[-1])
    in_range = (~(m_first | m_last)) & (v >= positions[0]) & (v <= positions[-1])

    coeff = np.zeros(v.shape + (Pn,), np.float32)
    coeff[..., 0] += m_first.astype(np.float32)
    coeff[..., Pn - 1] += m_last.astype(np.float32)
    for p in range(Pn - 1):
        m = (in_range & (v >= positions[p]) & (v < positions[p + 1])).astype(
            np.float32
        )
        t = v * varc[p] + const[p]
        coeff[..., p] += m * (1.0 - t)
        coeff[..., p + 1] += m * t

    Wk = np.transpose(weights, (0, 1, 3, 4, 2)).reshape(On, Cn, KHn * KWn, Pn)
    ident = np.all(np.abs(Wk - 1.0) <= np.float32(ATOL + RTOL), axis=-1)
    Wk_eff = np.where(ident[..., None], np.float32(0.0), Wk)

    out = np.einsum("bcklp,ockp->bol", coeff, Wk_eff, optimize=True)
    out = out + np.einsum(
        "bckl,ock->bol", v, ident.astype(np.float32), optimize=True
    )
    out = out + bias[None, :, None]
    return out.reshape(Bn, On, Hn, Wn).astype(np.float32)


def _host_weights(weights, bias):
    """Fold c3 and c2 away.  Returns (wall [128,6,O] bf16 lhsT stack,
    bias_eff [O,1] f32, ident_any).  Row layout matches Y's partition
    groups [v, g, v-shift, g-shift]; cols = [pair kw=0..2, singles]."""
    import ml_dtypes

    bf16 = ml_dtypes.bfloat16
    Wk = np.transpose(weights, (0, 1, 3, 4, 2)).reshape(O, C, K2, P)
    ident = np.all(np.abs(Wk - 1.0) <= np.float32(ATOL + RTOL), axis=-1)
    ident_any = bool(ident.any())
    Wk_eff = np.where(ident[..., None], np.float32(0.0), Wk).astype(np.float64)
    W2 = Wk_eff[:, :, :, 2]
    W3 = Wk_eff[:, :, :, 3]
    W4 = Wk_eff[:, :, :, 4]
    Ap = (2.0 * (W2 + W4 - 2.0 * W3)).astype(np.float32)   # [O,C,K2] g-weights
    Bw = (-2.0 * (W2 - W3)).astype(np.float32)             # [O,C,K2] v-weights
    # transpose to [C, K2, O] for lhsT rows
    ApT = np.ascontiguousarray(Ap.transpose(1, 2, 0))
    BwT = np.ascontiguousarray(Bw.transpose(1, 2, 0))

    def tap(kh, kw):
        return kh * KW + kw

    wall = np.zeros((128, 6, O), np.float32)
    for kw in range(KW):
        wall[0:32, kw] = BwT[:, tap(0, kw)]
        wall[32:64, kw] = ApT[:, tap(0, kw)]
        wall[64:96, kw] = BwT[:, tap(1, kw)]
        wall[96:128, kw] = ApT[:, tap(1, kw)]
    # col 3: singles (2,0) [rows 0-63] and (2,1) [rows 64-127, for ROWTILE]
    wall[0:32, 3] = BwT[:, tap(2, 0)]
    wall[32:64, 3] = ApT[:, tap(2, 0)]
    wall[64:96, 3] = BwT[:, tap(2, 1)]
    wall[96:128, 3] = ApT[:, tap(2, 1)]
    # col 4: single (2,2); col 5: single (2,1) fallback at rows 0-63
    wall[0:32, 4] = BwT[:, tap(2, 2)]
    wall[32:64, 4] = ApT[:, tap(2, 2)]
    wall[0:32, 5] = BwT[:, tap(2, 1)]
    wall[32:64, 5] = ApT[:, tap(2, 1)]

    bias_eff = (bias.astype(np.float64) + W2.sum(axis=(1, 2))).astype(np.float32)
    return (
        np.ascontiguousarray(wall.astype(bf16)),
        np.ascontiguousarray(bias_eff.reshape(O, 1)),
        ident_any,
    )


def _host_xg(x):
    """Build the full-SBUF-slot image of x: [ncores, 2 copies, C, IPC*YSZ]
    bf16, where each image slot is [lead0][66 rows of 64+gap][slack0] so a
    single DMA per partition group moves one contiguous run covering both
    images.  copy 0 = [pad; x; pad] rows (unshifted groups), copy 1 =
    [x; pad; pad] (row-shifted groups)."""
    import ml_dtypes

    WG = W + 1
    YSZ = 1 + (HP + 1) * WG
    xb16 = x.astype(ml_dtypes.bfloat16)
    rows = np.zeros((B, 2, C, HP, WG), dtype=ml_dtypes.bfloat16)
    rows[:, 0, :, 1 : H + 1, :W] = xb16
    rows[:, 1, :, 0:H, :W] = xb16
    slot = np.zeros((B, 2, C, YSZ), dtype=ml_dtypes.bfloat16)
    slot[:, :, :, 1 : 1 + HP * WG] = rows.reshape(B, 2, C, -1)
    # [B,2,C,YSZ] -> [ncores, 2, C, IPC, YSZ] -> flat per-partition runs
    slot = slot.reshape(NCORES, IPC, 2, C, YSZ).transpose(0, 2, 3, 1, 4)
    return np.ascontiguousarray(slot.reshape(NCORES, 2, C, IPC * YSZ))


# ---------------------------------------------------------------- device IR


def _build_nc():
    import concourse.tile as tile
    from concourse import bacc, mybir

    f32 = mybir.dt.float32
    bf16 = mybir.dt.bfloat16
    Alu = mybir.AluOpType
    Act = mybir.ActivationFunctionType

    WG = W + 1                   # row pitch: 64 data + 1 zero gap
    NIMG = H * WG                # flat elems per image plane (no pad rows)
    # Y flat layout: elem 0 = leading zero pad; logical row r at
    # [1 + r*WG, 1 + r*WG + W), gap at +W.  Unshifted groups: row 0 = top
    # pad, rows 1..H = image, row H+1 = bottom pad.  Shifted groups: row r
    # = image row r (i.e. shifted up by one), row H = bottom pad.
    YSZ = 1 + (HP + 1) * WG      # extra slack row so AP slices stay in bounds

    nc = bacc.Bacc("TRN2", target_bir_lowering=False, debug=False,
                   num_devices=NCORES)
    x_d = nc.dram_tensor("xg", [2, C, IPC * YSZ], bf16,
                         kind="ExternalInput").ap()
    w_d = nc.dram_tensor("wall", [128, 6, O], bf16, kind="ExternalInput").ap()
    b_d = nc.dram_tensor("bias", [O, 1], f32, kind="ExternalInput").ap()
    o_d = nc.dram_tensor("out", [IPC, O, H, W], bf16, kind="ExternalOutput").ap()

    with tile.TileContext(nc) as tc:
        with (
            tc.tile_pool(name="const", bufs=1) as constp,
            tc.tile_pool(name="ybuf", bufs=1) as ybufp,
            tc.tile_pool(name="psum", bufs=1, space="PSUM") as psump,
            tc.tile_pool(name="osb", bufs=4) as osbp,
        ):
            # both images side by side per partition: one contiguous
            # 17.4KB-per-partition run per x load (packet-rate relief)
            YB = ybufp.tile([128, IPC * YSZ], bf16, name="YB")

            # ---- x loads: one DMA per partition group (both images) ----
            # phi-gating groups (g1 = c4-in, g3 = c4-in shifted) first
            nc.sync.dma_start(YB[32:64, :], x_d[0])
            nc.scalar.dma_start(YB[96:128, :], x_d[1])
            w_sb = constp.tile([128, 6, O], bf16)
            nc.scalar.dma_start(w_sb[:], w_d[:])
            nc.sync.dma_start(YB[0:32, :], x_d[0])
            nc.scalar.dma_start(YB[64:96, :], x_d[1])
            b_sb = constp.tile([O, 1], f32)
            nc.sync.dma_start(b_sb[:], b_d[:])

            # pull the ACT table load off the critical path
            tiny = constp.tile([C, 1], f32)
            nc.gpsimd.memset(tiny[:], 0.0)
            nc.scalar.activation(tiny[:], tiny[:], Act.Relu, bias=0.0, scale=1.0)

            # PE warmup: dummy matmuls bridge to the real stream (a cold PE
            # runs at 1.2GHz for the first ~3.4us of sustained activity)
            zb = constp.tile([128, 512], bf16)
            nc.gpsimd.memset(zb[:], 0.0)
            pw = psump.tile([O, 512], f32, name="ps_warm", tag="ps0")
            for j in range(9):
                nc.tensor.matmul(
                    pw[:], zb[:, 0:128], zb[:],
                    start=(j == 0), stop=(j == 8),
                )

            # ---- g = relu(v - 0.5) in place on partitions 32:64, 96:128 --
            # all on vector (DVE ~3x scalar for bf16 elementwise); phase 0
            # covers rows [0,35) which unblocks tiles 0-3
            def phi(i, phase):
                a, b = (0, 35) if phase == 0 else (35, HP)
                for p0 in (32, 96):
                    ap = YB[p0 : p0 + 32,
                            i * YSZ + 1 + a * WG : i * YSZ + 1 + b * WG]
                    nc.vector.tensor_scalar(
                        ap, ap, 0.5, 0.0, Alu.subtract, Alu.max,
                    )

            for i in range(IPC):
                phi(i, 0)
                phi(i, 1)

            def ywin(i, p0, p1, r0, kw):
                """[p1-p0, RT, W] window: rows r0..r0+RT, cols kw-1..kw-1+W."""
                off = i * YSZ + 1 + r0 * WG + (kw - 1)
                return YB[p0:p1, off : off + RT * WG].rearrange(
                    "p (r c) -> p r c", r=RT
                )[:, :, 0:W]

            # ---- GEMM: 2 half-batches of 4 tiles per image; singles
            # first (they need only partitions 0:64, which land first) ----
            def mm_half(i, half, pss):
                ts = range(4 * half, 4 * half + 4)
                for t in ts:
                    nc.tensor.matmul(
                        pss[t][:], w_sb[0:64, 3, :],
                        ywin(i, 0, 64, t * RT + 2, 0),
                        start=True, stop=False,
                    )
                    if ROWTILE:
                        nc.tensor.matmul(
                            pss[t][:], w_sb[64:128, 3, :],
                            ywin(i, 64, 128, t * RT + 1, 1),
                            start=False, stop=False,
                        )
                    else:
                        nc.tensor.matmul(
                            pss[t][:], w_sb[0:64, 5, :],
                            ywin(i, 0, 64, t * RT + 2, 1),
                            start=False, stop=False,
                        )
                for t in ts:
                    nc.tensor.matmul(
                        pss[t][:], w_sb[0:64, 4, :],
                        ywin(i, 0, 64, t * RT + 2, 2),
                        start=False, stop=False,
                    )
                for kw in range(KW):
                    for t in ts:
                        nc.tensor.matmul(
                            pss[t][:], w_sb[:, kw, :],
                            ywin(i, 0, 128, t * RT, kw),
                            start=False, stop=(kw == KW - 1),
                        )
                # drain tile pairs into one buffer -> one 2KB-run out-DMA
                for tp in (ts[0], ts[2]):
                    osb = osbp.tile([O, 2 * RT * W], bf16, name="osb")
                    nc.scalar.activation(
                        osb[:, 0 : RT * W], pss[tp][:], Act.Identity,
                        bias=b_sb[:, 0:1], scale=1.0,
                    )
                    nc.vector.tensor_scalar(
                        osb[:, RT * W : 2 * RT * W], pss[tp + 1][:],
                        b_sb[:, 0:1], None, Alu.add,
                    )
                    eng = nc.sync if tp % 4 == 0 else nc.scalar
                    eng.dma_start(
                        o_d[i, :, tp * RT : (tp + 2) * RT, :],
                        osb[:].rearrange("o (r w) -> o r w", r=2 * RT),
                    )

            for i in range(IPC):
                pss = [
                    psump.tile([O, RT * W], f32, name=f"ps_i{i}t{t}",
                               tag=f"ps{t}")
                    for t in range(NT)
                ]
                mm_half(i, 0, pss)
                mm_half(i, 1, pss)
    nc.compile()
    return nc


# ---------------------------------------------------------------- entry


def _prep(inputs):
    x = np.ascontiguousarray(np.asarray(inputs["x"], dtype=np.float32))
    weights = np.ascontiguousarray(np.asarray(inputs["weights"], dtype=np.float32))
    bias = np.ascontiguousarray(np.asarray(inputs["bias"], dtype=np.float32))
    positions = np.ascontiguousarray(
        np.asarray(inputs["positions"], dtype=np.float32)
    )
    return x, weights, bias, positions


def _fast_path_ok(x, positions):
    expect = np.linspace(-1.0, 1.0, P, dtype=np.float32)
    return (
        x.shape == (B, C, H, W)
        and positions.shape == (P,)
        and np.array_equal(positions, expect)
        and float(x.min()) >= 0.0
        and float(x.max()) <= 1.0
    )


def kernel(**inputs):
    import ml_dtypes

    x, weights, bias, positions = _prep(inputs)
    if not _fast_path_ok(x, positions):
        return _reference_np(x, weights, bias, positions)

    wall, bias_eff, ident_any = _host_weights(weights, bias)
    if ident_any:
        # identity-shortcut weights present: needs the raw-v plane; use the
        # exact fallback rather than a rarely-exercised device path
        return _reference_np(x, weights, bias, positions)

    from concourse.bass_utils import run_bass_kernel_spmd

    nc = _build_nc()
    # copy 0: [pad; x; pad] rows 0..65 for the unshifted groups; copy 1:
    # [x; pad; pad] for the row-shifted groups; each duplicated over 64
    # channels so one DMA fills two 32-partition groups.  col W is the
    # zero gap that doubles as left/right conv padding.
    xg = _host_xg(x)
    in_maps = [
        {"xg": xg[i], "wall": wall, "bias": bias_eff}
        for i in range(NCORES)
    ]
    res = run_bass_kernel_spmd(nc, in_maps, core_ids=list(range(NCORES)))
    out = np.concatenate([res.results[i]["out"] for i in range(NCORES)], axis=0)
    return np.ascontiguousarray(out.astype(np.float32))


# ------------------------------------------------------------ dev utilities


def _run_sim(inputs):
    """CoreSim single-core run (images 0..IPC-1) for correctness debugging."""
    import ml_dtypes
    from concourse.bass_interp import CoreSim

    x, weights, bias, positions = _prep(inputs)
    assert _fast_path_ok(x, positions)
    wall, bias_eff, ident_any = _host_weights(weights, bias)
    assert not ident_any
    nc = _build_nc()
    sim = CoreSim(nc)
    sim.tensor("xg")[:] = _host_xg(
        np.broadcast_to(x[:IPC], (NCORES, IPC) + x.shape[1:]).reshape(
            B, *x.shape[1:]
        )
    )[0]
    sim.tensor("wall")[:] = wall
    sim.tensor("bias")[:] = bias_eff
    sim.simulate()
    return np.array(sim.tensor("out")).astype(np.float32)
